# revision 5
# baseline (speedup 1.0000x reference)
"""Bass/Trainium2 kernel for nn_Attention_6983616824195.

Single-head attention with Dense projections:
    q = Q @ WQ ; k = K @ WK ; v = V @ WV        (B, L, 128)
    S = q @ k^T ; S = where(mask==1, S, -inf) ; S /= sqrt(128)
    out = softmax(S, axis=-1) @ v               (B, L, 128)

Shapes: B=4, L=4096, DM=1024, DK=DV=128, mask [B, 1, L] (key mask).

Sharding: 8 cores = (batch b, query-half h). Core c = (b=c//2, h=c%2)
computes queries [h*2048, (h+1)*2048) of batch b against the full key
set of batch b. K/V projections are recomputed on both half-cores of a
batch (cheap vs. collectives).

Per-core dataflow (all matmuls contract over the SBUF partition dim):
  - Host supplies QT/KT/VT pre-transposed to [dm, seq] in bf16, so no
    on-chip transposes are needed anywhere.
  - kT[d, s]  = sum_c WK[c]^T·KT[c]        (lhsT=WK chunk, rhs=KT chunk)
  - qT[d, q]  = sum_c WQ[c]^T·QT[c]
  - v[s, dv]  = sum_c VT[c]^T·WV[c]        (lhsT=VT tile, rhs=WV chunk)
  - vext[s, 0:128] = v * mask[s]; vext[s, 128] = mask[s]  (ones column)
  - S^T[s, q] = kT^T·qT   (lhsT=kT s-tile, rhs=qT q-block, psum f32)
  - e = exp(S^T * 1/sqrt(128))  (ScalarE, bf16 out; no max-subtraction:
    logits are ~N(0,1) after scaling, |logit| < ~7, exp cannot overflow)
  - A[q, 0:129] = sum_s e^T·vext  (lhsT=e tile, rhs=vext tile) — column
    128 is the softmax denominator sum_s mask[s]*e[s,q].
  - out[q, dv] = A[:, 0:128] * (1 / A[:, 128])
Masking is exact: masked keys get weight mask[s]=0 in both numerator
and denominator, identical to where(mask==1, S, -inf) softmax.
"""

import numpy as np
import ml_dtypes

import concourse.bass as bass
import concourse.tile as tile
import concourse.mybir as mybir
from concourse.bass_utils import run_bass_kernel_spmd

B, L, DM = 4, 4096, 1024
DK = DV = 128
N_CORES = 8
LQ = L // 2            # queries per core (2048)
P = 128
NDC = DM // P          # dm chunks (8)
NQB = LQ // 512        # q blocks of 512 (4)
NQT_PER_B = 512 // P   # q tiles per block (4)
NST = L // P           # s tiles (32)
NSB = L // 512         # s blocks of 512 (8)
VW = DV + 1            # v-ext width (129): 128 dv cols + ones column
SCALE = 1.0 / float(np.sqrt(DK))

F32 = mybir.dt.float32
BF16 = mybir.dt.bfloat16


def _split_drain_waits(nc, max_waits=1):
    """This walrus build encodes at most one sync-wait per instruction;
    move surplus waits onto preceding NoOps on the same engine."""
    for f in nc.m.functions:
        for bb in f.blocks:
            new_insts = []
            for inst in bb.instructions:
                si = inst.sync_info
                if si is not None and si.on_wait and len(si.on_wait) > max_waits:
                    waits = list(si.on_wait)
                    extra, keep = waits[:-max_waits], waits[-max_waits:]
                    for k, w in enumerate(extra):
                        nop = mybir.InstNoOp(name=f"{inst.name}_wsplit{k}")
                        nop.engine = inst.engine
                        nop.sync_info = mybir.SyncInfo(on_wait=[w], on_update=[])
                        new_insts.append(nop)
                    inst.sync_info = mybir.SyncInfo(
                        on_wait=keep, on_update=list(si.on_update)
                    )
                new_insts.append(inst)
            bb.instructions = new_insts


def build_nc():
    nc = bass.Bass("TRN2", target_bir_lowering=False, debug=False)

    qt_d = nc.dram_tensor("QT", [DM, LQ], BF16, kind="ExternalInput").ap()
    kt_d = nc.dram_tensor("KT", [DM, L], BF16, kind="ExternalInput").ap()
    vt_d = nc.dram_tensor("VT", [DM, L], BF16, kind="ExternalInput").ap()
    wq_d = nc.dram_tensor("WQ", [DM, DK], BF16, kind="ExternalInput").ap()
    wk_d = nc.dram_tensor("WK", [DM, DK], BF16, kind="ExternalInput").ap()
    wv_d = nc.dram_tensor("WV", [DM, DV], BF16, kind="ExternalInput").ap()
    mk_d = nc.dram_tensor("MK", [L, 1], F32, kind="ExternalInput").ap()
    o_d = nc.dram_tensor("O", [LQ, DV], F32, kind="ExternalOutput").ap()

    with tile.TileContext(nc) as tc:
        from contextlib import ExitStack

        with ExitStack() as ctx:
            # ---- persistent SBUF pools ----
            wpool = ctx.enter_context(tc.tile_pool(name="w", bufs=1))
            kqv = ctx.enter_context(tc.tile_pool(name="kqv", bufs=1))
            mpool = ctx.enter_context(tc.tile_pool(name="mk", bufs=4))
            vraw_pool = ctx.enter_context(tc.tile_pool(name="vraw", bufs=4))
            epool = ctx.enter_context(tc.tile_pool(name="e", bufs=6))
            fin = ctx.enter_context(tc.tile_pool(name="fin", bufs=4))
            # raw K/Q input pool (released after projections)
            raw = ctx.enter_context(tc.tile_pool(name="raw", bufs=6))

            # ---- PSUM: pv persistent; pk released before ps/pav ----
            pv = ctx.enter_context(tc.tile_pool(name="pv", bufs=2, space="PSUM"))

            # ---- load weights: [128, 8*128], chunk c at cols [c*128,(c+1)*128) ----
            wq = wpool.tile([P, NDC * DK], BF16)
            wk = wpool.tile([P, NDC * DK], BF16)
            wv = wpool.tile([P, NDC * DV], BF16)
            for c in range(NDC):
                nc.sync.dma_start(wq[:, c * DK : (c + 1) * DK], wq_d[c * P : (c + 1) * P, :])
                nc.sync.dma_start(wk[:, c * DK : (c + 1) * DK], wk_d[c * P : (c + 1) * P, :])
                nc.sync.dma_start(wv[:, c * DV : (c + 1) * DV], wv_d[c * P : (c + 1) * P, :])

            # ---- persistent projected tensors ----
            kT = kqv.tile([P, L], BF16)        # [d, s]
            qT = kqv.tile([P, LQ], BF16)       # [d, q]
            vext = kqv.tile([P, NST * VW], BF16)  # per s-tile: [s, 129]

            with tc.tile_pool(name="pk", bufs=2, space="PSUM") as pk:
                # ---- k projection: kT[:, sb*512:(sb+1)*512] ----
                for sb in range(NSB):
                    kr = raw.tile([P, NDC * 512], BF16, tag="kraw")
                    for c in range(NDC):
                        nc.sync.dma_start(
                            kr[:, c * 512 : (c + 1) * 512],
                            kt_d[c * P : (c + 1) * P, sb * 512 : (sb + 1) * 512],
                        )
                    psk = pk.tile([P, 512], F32, tag="pproj")
                    for c in range(NDC):
                        nc.tensor.matmul(
                            psk[:],
                            wk[:, c * DK : (c + 1) * DK],
                            kr[:, c * 512 : (c + 1) * 512],
                            start=(c == 0),
                            stop=(c == NDC - 1),
                        )
                    nc.vector.tensor_copy(kT[:, sb * 512 : (sb + 1) * 512], psk[:])

                # ---- q projection ----
                for qb in range(NQB):
                    qr = raw.tile([P, NDC * 512], BF16, tag="kraw")
                    for c in range(NDC):
                        nc.sync.dma_start(
                            qr[:, c * 512 : (c + 1) * 512],
                            qt_d[c * P : (c + 1) * P, qb * 512 : (qb + 1) * 512],
                        )
                    psq = pk.tile([P, 512], F32, tag="pproj")
                    for c in range(NDC):
                        nc.tensor.matmul(
                            psq[:],
                            wq[:, c * DK : (c + 1) * DK],
                            qr[:, c * 512 : (c + 1) * 512],
                            start=(c == 0),
                            stop=(c == NDC - 1),
                        )
                    nc.vector.tensor_copy(qT[:, qb * 512 : (qb + 1) * 512], psq[:])

            # ---- v projection for s-tile j (interleaved into first q-block) ----
            def vproj(j):
                vr = vraw_pool.tile([P, NDC * P], BF16, tag="vraw")
                for c in range(NDC):
                    nc.sync.dma_start(
                        vr[:, c * P : (c + 1) * P],
                        vt_d[c * P : (c + 1) * P, j * P : (j + 1) * P],
                    )
                mkt = mpool.tile([P, 1], F32, tag="mk")
                nc.sync.dma_start(mkt[:], mk_d[j * P : (j + 1) * P, :])
                psv = pv.tile([P, DV], F32)
                for c in range(NDC):
                    nc.tensor.matmul(
                        psv[:],
                        vr[:, c * P : (c + 1) * P],
                        wv[:, c * DV : (c + 1) * DV],
                        start=(c == 0),
                        stop=(c == NDC - 1),
                    )
                # vext cols 0:128 = v * mask (per-partition scalar), col 128 = mask
                nc.scalar.activation(
                    vext[:, j * VW : j * VW + DV],
                    psv[:],
                    mybir.ActivationFunctionType.Copy,
                    scale=mkt[:],
                )
                nc.scalar.copy(vext[:, j * VW + DV : j * VW + VW], mkt[:])

            # ---- attention over q-blocks ----
            with (
                tc.tile_pool(name="ps", bufs=2, space="PSUM") as ps,
                tc.tile_pool(name="pav", bufs=4, space="PSUM") as pav,
            ):
                for qb in range(NQB):
                    avp = [
                        pav.tile([P, VW], F32, tag="av", name=f"avp_qb{qb}_t{t}")
                        for t in range(NQT_PER_B)
                    ]
                    for j in range(NST):
                        if qb == 0:
                            vproj(j)
                        pss = ps.tile([P, 512], F32)
                        nc.tensor.matmul(
                            pss[:],
                            kT[:, j * P : (j + 1) * P],
                            qT[:, qb * 512 : (qb + 1) * 512],
                            start=True,
                            stop=True,
                        )
                        et = epool.tile([P, 512], BF16, tag="e")
                        nc.scalar.activation(
                            et[:], pss[:], mybir.ActivationFunctionType.Exp, scale=SCALE
                        )
                        for t in range(NQT_PER_B):
                            nc.tensor.matmul(
                                avp[t][:],
                                et[:, t * P : (t + 1) * P],
                                vext[:, j * VW : (j + 1) * VW],
                                start=(j == 0),
                                stop=(j == NST - 1),
                            )
                    for t in range(NQT_PER_B):
                        dinv = fin.tile([P, 1], F32, tag="dinv")
                        nc.vector.reciprocal(dinv[:], avp[t][:, DV : DV + 1])
                        osb = fin.tile([P, DV], F32, tag="osb")
                        nc.vector.tensor_scalar_mul(osb[:], avp[t][:, 0:DV], dinv[:])
                        q0 = qb * 512 + t * P
                        nc.sync.dma_start(o_d[q0 : q0 + P, :], osb[:])

    _split_drain_waits(nc)
    return nc


_NC = None


def _get_nc():
    global _NC
    if _NC is None:
        _NC = build_nc()
    return _NC


def make_in_maps(Q, K, V, mask, WQ, WK, WV):
    bf = ml_dtypes.bfloat16
    Q = np.asarray(Q, dtype=np.float32)
    K = np.asarray(K, dtype=np.float32)
    V = np.asarray(V, dtype=np.float32)
    mask = np.asarray(mask)
    wq = np.ascontiguousarray(np.asarray(WQ, dtype=np.float32).astype(bf))
    wk = np.ascontiguousarray(np.asarray(WK, dtype=np.float32).astype(bf))
    wv = np.ascontiguousarray(np.asarray(WV, dtype=np.float32).astype(bf))
    in_maps = []
    for c in range(N_CORES):
        b, h = c // 2, c % 2
        qt = np.ascontiguousarray(Q[b, h * LQ : (h + 1) * LQ, :].astype(bf).T)
        kt = np.ascontiguousarray(K[b].astype(bf).T)
        vt = np.ascontiguousarray(V[b].astype(bf).T)
        mk = (mask[b, 0, :] == 1).astype(np.float32).reshape(L, 1)
        in_maps.append(
            {"QT": qt, "KT": kt, "VT": vt, "WQ": wq, "WK": wk, "WV": wv, "MK": mk}
        )
    return in_maps


def assemble(results):
    out = np.empty((B, L, DV), dtype=np.float32)
    for c in range(N_CORES):
        b, h = c // 2, c % 2
        out[b, h * LQ : (h + 1) * LQ, :] = results[c]["O"]
    return out


def kernel(Q, K, V, mask, WQ, WK, WV):
    in_maps = make_in_maps(Q, K, V, mask, WQ, WK, WV)
    res = run_bass_kernel_spmd(_get_nc(), in_maps, core_ids=list(range(N_CORES)))
    return assemble(res.results)


# revision 11
# speedup vs baseline: 1.7769x; 1.7769x over previous
"""Bass/Trainium2 kernel for nn_Attention_6983616824195.

Single-head attention with Dense projections:
    q = Q @ WQ ; k = K @ WK ; v = V @ WV        (B, L, 128)
    S = q @ k^T ; S = where(mask==1, S, -inf) ; S /= sqrt(128)
    out = softmax(S, axis=-1) @ v               (B, L, 128)

Shapes: B=4, L=4096, DM=1024, DK=DV=128, mask [B, 1, L] (key mask).

Sharding: 8 cores = (batch b, query-half h). Core c = (b=c//2, h=c%2)
computes queries [h*2048, (h+1)*2048) of batch b against the full key
set of batch b. K/V projections are recomputed on both half-cores of a
batch (cheap vs. collectives); WQ/WK/WV are replicated.

Per-core dataflow (all matmuls contract over the SBUF partition dim):
  - Host supplies Q/K/V in a dm-blocked transposed bf16 layout so every
    DMA is a single instruction whose per-partition segments are 2-8KB
    contiguous (HWDGE/descriptor-bound otherwise), and no on-chip
    transposes are needed anywhere.
  - kT[d, s]  = sum_c WK[c]^T·KT[c]        (lhsT=WK chunk, rhs=KT chunk)
  - qT[d, q]  = sum_c WQ[c]^T·QT[c]
  - v[s, dv]  = sum_c VT[c]^T·WV[c]        (lhsT=VT tile, rhs=WV chunk)
  - vext[s, 0:128] = v * mask[s]; vext[s, 128] = mask[s]  (ones column)
  - S^T[s, q] = kT^T·qT   (lhsT=kT s-tile, rhs=qT q-block, psum f32,
    two s-tiles paired into one [128, 1024] psum tile)
  - e = exp(S^T / sqrt(128))  (one ScalarE op per pair, bf16 out; no
    max-subtraction: logits are ~N(0,1) after scaling, exp can't
    overflow for this problem's data distribution)
  - A[q, 0:129] = sum_s e^T·vext  (lhsT=e tile, rhs=vext tile) — column
    128 is the softmax denominator sum_s mask[s]*e[s,q].
  - out[q, dv] = A[:, 0:128] * (1 / A[:, 128])
Masking is exact: masked keys get weight mask[s]=0 in both numerator
and denominator, identical to where(mask==1, S, -inf) softmax.
"""

import numpy as np
import ml_dtypes

import concourse.bass as bass
import concourse.tile as tile
import concourse.mybir as mybir
from concourse.bass_utils import run_bass_kernel_spmd

B, L, DM = 4, 4096, 1024
DK = DV = 128
N_CORES = 8
LQ = L // 2            # queries per core (2048)
P = 128
NDC = DM // P          # dm chunks (8)
NQB = LQ // 512        # q blocks of 512 (4)
NQT_PER_B = 512 // P   # q tiles per block (4)
NST = L // P           # s tiles (32)
NSP = NST // 2         # s-tile pairs (16)
NSB = L // 512         # s blocks of 512 (8)
VW = DV + 1            # v-ext width (129): 128 dv cols + ones column
SCALE = 1.0 / float(np.sqrt(DK))

F32 = mybir.dt.float32
BF16 = mybir.dt.bfloat16


def _split_multi_waits(nc, max_waits=1):
    """This walrus build encodes at most one sync-wait per instruction;
    move surplus waits onto preceding NoOps on the same engine."""
    for f in nc.m.functions:
        for bb in f.blocks:
            new_insts = []
            for inst in bb.instructions:
                si = inst.sync_info
                if si is not None and si.on_wait and len(si.on_wait) > max_waits:
                    waits = list(si.on_wait)
                    extra, keep = waits[:-max_waits], waits[-max_waits:]
                    for k, w in enumerate(extra):
                        nop = mybir.InstNoOp(name=f"{inst.name}_wsplit{k}")
                        nop.engine = inst.engine
                        nop.sync_info = mybir.SyncInfo(on_wait=[w], on_update=[])
                        new_insts.append(nop)
                    inst.sync_info = mybir.SyncInfo(
                        on_wait=keep, on_update=list(si.on_update)
                    )
                new_insts.append(inst)
            bb.instructions = new_insts


def build_nc(split_waits=True):
    nc = bass.Bass("TRN2", target_bir_lowering=False, debug=False)

    # Host-blocked layouts (see make_in_maps):
    #   QTB[qb*128+p, c*512+u] = Q[b, h*2048 + qb*512+u, c*128+p]
    #   KTB[sb*128+p, c*512+u] = K[b, sb*512+u, c*128+p]
    #   VTB[j*128+p,  c*128+q] = V[b, j*128+q,  c*128+p]
    #   WxB[p, c*128+k]        = Wx[c*128+p, k]
    #   MKB[p, j]              = (mask[b, 0, j*128+p] == 1)
    qt_d = nc.dram_tensor("QTB", [NQB * P, NDC * 512], BF16, kind="ExternalInput").ap()
    kt_d = nc.dram_tensor("KTB", [NSB * P, NDC * 512], BF16, kind="ExternalInput").ap()
    vt_d = nc.dram_tensor("VTB", [NST * P, NDC * P], BF16, kind="ExternalInput").ap()
    wq_d = nc.dram_tensor("WQB", [P, NDC * DK], BF16, kind="ExternalInput").ap()
    wk_d = nc.dram_tensor("WKB", [P, NDC * DK], BF16, kind="ExternalInput").ap()
    wv_d = nc.dram_tensor("WVB", [P, NDC * DV], BF16, kind="ExternalInput").ap()
    mk_d = nc.dram_tensor("MKB", [P, NST], F32, kind="ExternalInput").ap()
    o_d = nc.dram_tensor("O", [LQ, DV], F32, kind="ExternalOutput").ap()

    with tile.TileContext(nc) as tc:
        from contextlib import ExitStack

        with ExitStack() as ctx:
            # ---- persistent SBUF pools ----
            wpool = ctx.enter_context(tc.tile_pool(name="w", bufs=1))
            kqv = ctx.enter_context(tc.tile_pool(name="kqv", bufs=1))
            vraw_pool = ctx.enter_context(tc.tile_pool(name="vraw", bufs=4))
            epool = ctx.enter_context(tc.tile_pool(name="e", bufs=4))
            fin = ctx.enter_context(tc.tile_pool(name="fin", bufs=4))
            raw = ctx.enter_context(tc.tile_pool(name="raw", bufs=6))

            # ---- load weights + mask (one DMA each) ----
            wq = wpool.tile([P, NDC * DK], BF16)
            wk = wpool.tile([P, NDC * DK], BF16)
            wv = wpool.tile([P, NDC * DV], BF16)
            mkb = wpool.tile([P, NST], F32)
            nc.sync.dma_start(wq[:], wq_d[:])
            nc.sync.dma_start(wk[:], wk_d[:])
            nc.sync.dma_start(wv[:], wv_d[:])
            nc.sync.dma_start(mkb[:], mk_d[:])

            # ---- persistent projected tensors ----
            kT = kqv.tile([P, L], BF16)           # [d, s]
            qT = kqv.tile([P, LQ], BF16)          # [d, q]
            vext = kqv.tile([P, NST * VW], BF16)  # per s-tile: [s, 129]

            with tc.tile_pool(name="pk", bufs=2, space="PSUM") as pk:
                # ---- k projection: kT[:, sb*512:(sb+1)*512] ----
                for sb in range(NSB):
                    kr = raw.tile([P, NDC * 512], BF16, tag="kraw")
                    nc.sync.dma_start(kr[:], kt_d[sb * P : (sb + 1) * P, :])
                    psk = pk.tile([P, 512], F32, tag="pproj")
                    for c in range(NDC):
                        nc.tensor.matmul(
                            psk[:],
                            wk[:, c * DK : (c + 1) * DK],
                            kr[:, c * 512 : (c + 1) * 512],
                            start=(c == 0),
                            stop=(c == NDC - 1),
                        )
                    nc.vector.tensor_copy(kT[:, sb * 512 : (sb + 1) * 512], psk[:])

                # ---- q projection ----
                for qb in range(NQB):
                    qr = raw.tile([P, NDC * 512], BF16, tag="kraw")
                    nc.sync.dma_start(qr[:], qt_d[qb * P : (qb + 1) * P, :])
                    psq = pk.tile([P, 512], F32, tag="pproj")
                    for c in range(NDC):
                        nc.tensor.matmul(
                            psq[:],
                            wq[:, c * DK : (c + 1) * DK],
                            qr[:, c * 512 : (c + 1) * 512],
                            start=(c == 0),
                            stop=(c == NDC - 1),
                        )
                    nc.vector.tensor_copy(qT[:, qb * 512 : (qb + 1) * 512], psq[:])

            # ---- v projection for s-tile j (interleaved into first q-block) ----
            def make_vproj(pv):
                def vproj(j):
                    vr = vraw_pool.tile([P, NDC * P], BF16, tag="vraw")
                    nc.sync.dma_start(vr[:], vt_d[j * P : (j + 1) * P, :])
                    psv = pv.tile([P, DV], F32, tag="psv")
                    for c in range(NDC):
                        nc.tensor.matmul(
                            psv[:],
                            vr[:, c * P : (c + 1) * P],
                            wv[:, c * DV : (c + 1) * DV],
                            start=(c == 0),
                            stop=(c == NDC - 1),
                        )
                    # vext cols 0:128 = v*mask (per-partition), col 128 = mask
                    nc.vector.tensor_scalar_mul(
                        vext[:, j * VW : j * VW + DV], psv[:], mkb[:, j : j + 1]
                    )
                    nc.vector.tensor_copy(
                        vext[:, j * VW + DV : j * VW + VW], mkb[:, j : j + 1]
                    )
                return vproj

            # ---- attention ----
            def qblock(qb, ps, pav, vproj, ps_width):
                avp = [
                    pav.tile([P, VW], F32, tag="av", name=f"avp_qb{qb}_t{t}")
                    for t in range(NQT_PER_B)
                ]
                pairs = ps_width // 512  # s-tiles per psum tile (1 or 2)
                for jp in range(NST // pairs):
                    pss = ps.tile([P, ps_width], F32, tag="pss", name=f"pss_{qb}_{jp}")
                    for u in range(pairs):
                        j = jp * pairs + u
                        if vproj is not None:
                            vproj(j)
                        nc.tensor.matmul(
                            pss[:, u * 512 : (u + 1) * 512],
                            kT[:, j * P : (j + 1) * P],
                            qT[:, qb * 512 : (qb + 1) * 512],
                            start=True,
                            stop=True,
                        )
                    et = epool.tile([P, ps_width], BF16, tag="e", name=f"et_{qb}_{jp}")
                    nc.scalar.activation(
                        et[:], pss[:], mybir.ActivationFunctionType.Exp, scale=SCALE
                    )
                    for u in range(pairs):
                        j = jp * pairs + u
                        for t in range(NQT_PER_B):
                            nc.tensor.matmul(
                                avp[t][:],
                                et[:, u * 512 + t * P : u * 512 + (t + 1) * P],
                                vext[:, j * VW : (j + 1) * VW],
                                start=(j == 0),
                                stop=(j == NST - 1),
                            )
                for t in range(NQT_PER_B):
                    dinv = fin.tile([P, 1], F32, tag="dinv", name=f"dinv{qb}_{t}")
                    nc.vector.reciprocal(dinv[:], avp[t][:, DV : DV + 1])
                    osb = fin.tile([P, DV], F32, tag="osb", name=f"osb{qb}_{t}")
                    nc.vector.tensor_scalar_mul(osb[:], avp[t][:, 0:DV], dinv[:])
                    q0 = qb * 512 + t * P
                    nc.sync.dma_start(o_d[q0 : q0 + P, :], osb[:])

            with tc.tile_pool(name="pav", bufs=4, space="PSUM") as pav:
                # q-block 0: v-projection interleaved; single-buffered scores
                with (
                    tc.tile_pool(name="pv", bufs=1, space="PSUM") as pv,
                    tc.tile_pool(name="ps0", bufs=1, space="PSUM") as ps0,
                ):
                    qblock(0, ps0, pav, make_vproj(pv), 1024)
                # q-blocks 1..3: double-buffered paired scores
                with tc.tile_pool(name="ps", bufs=2, space="PSUM") as ps:
                    for qb in range(1, NQB):
                        qblock(qb, ps, pav, None, 1024)

    if split_waits:
        _split_multi_waits(nc)
    return nc


_NC = None


def _get_nc():
    global _NC
    if _NC is None:
        _NC = build_nc()
    return _NC


def _block2(x, rows):
    """x [S, DM] -> blocked [S//rows * P, NDC*rows]:
    out[blk*P + p, c*rows + u] = x[blk*rows + u, c*P + p]"""
    S = x.shape[0]
    nblk = S // rows
    r = x.reshape(nblk, rows, NDC, P)
    return np.ascontiguousarray(r.transpose(0, 3, 2, 1)).reshape(nblk * P, NDC * rows)


def make_in_maps(Q, K, V, mask, WQ, WK, WV):
    bf = ml_dtypes.bfloat16
    Q = np.asarray(Q, dtype=np.float32)
    K = np.asarray(K, dtype=np.float32)
    V = np.asarray(V, dtype=np.float32)
    mask = np.asarray(mask)

    def wblock(W):
        w = np.asarray(W, dtype=np.float32).astype(bf)
        return np.ascontiguousarray(w.reshape(NDC, P, DK).transpose(1, 0, 2)).reshape(
            P, NDC * DK
        )

    wqb, wkb, wvb = wblock(WQ), wblock(WK), wblock(WV)

    per_batch = []
    for b in range(B):
        ktb = _block2(K[b].astype(bf), 512)
        vtb = _block2(V[b].astype(bf), P)
        mkb = np.ascontiguousarray(
            (mask[b, 0, :] == 1).astype(np.float32).reshape(NST, P).T
        )
        per_batch.append((ktb, vtb, mkb))

    in_maps = []
    for c in range(N_CORES):
        b, h = c // 2, c % 2
        ktb, vtb, mkb = per_batch[b]
        qtb = _block2(Q[b, h * LQ : (h + 1) * LQ, :].astype(bf), 512)
        in_maps.append(
            {
                "QTB": qtb,
                "KTB": ktb,
                "VTB": vtb,
                "WQB": wqb,
                "WKB": wkb,
                "WVB": wvb,
                "MKB": mkb,
            }
        )
    return in_maps


def assemble(results):
    out = np.empty((B, L, DV), dtype=np.float32)
    for c in range(N_CORES):
        b, h = c // 2, c % 2
        out[b, h * LQ : (h + 1) * LQ, :] = results[c]["O"]
    return out


def kernel(Q, K, V, mask, WQ, WK, WV):
    in_maps = make_in_maps(Q, K, V, mask, WQ, WK, WV)
    res = run_bass_kernel_spmd(_get_nc(), in_maps, core_ids=list(range(N_CORES)))
    return assemble(res.results)
